# revision 43
# baseline (speedup 1.0000x reference)
"""ArcFace (AngularPenaltySMLoss) on 8 TRN2 NeuronCores. ~190 us HW exec.

Sharding (model-parallel softmax, per the classic ArcFace recipe):
  - The 32768 classes are sharded across the 8 cores (4096 each).
  - Host prep is layout/dtype only: features are transposed to fT [512,2048]
    fp8e4m3 (raw values), each weight shard is transposed to wT [512,4096]
    bf16, and the target rows wtgt = weight[y_true] are gathered (pure
    indexing) as bf16. No arithmetic happens on the host.

Per core (all cores run the identical SPMD graph):
  - Weight-column norms: squares (DVE bf16 2x) -> ones-vector matmul
    partition-sum (PE) -> 1/sqrt as exp(-0.5*ln(x)) on ACT (both functions
    live in the single natural_log_exp table set; a monkeypatch maps every
    activation to that set so the table is loaded exactly once) -> K=1
    matmul broadcast across partitions -> what = wT * bcast, quantized to
    fp8. PSUM scratch for these borrows main-loop z-pool slots, so there is
    no pool barrier between prep and the main loop.
  - Feature normalization costs nothing: the matmul output keeps batch on
    the partition axis, so 64/||f_b|| (from an ACT Square+reduce of the
    natural-layout features) is applied as the per-partition `scale` of the
    main-loop Exp. Raw fp8 features feed the matmul directly.
  - Main loop (class-half-major so it starts once 4 of 8 chunks are ready):
    z = fT^T @ what accumulated over K=512 in PSUM via fp8 DoubleRow
    matmuls (2 fp8 MACs/cell/cycle); ACT Exp in place on PSUM [128,2048]
    with scale=64/||f|| and accum_out -> per-row partial exp sums. The full
    exp matrix is never materialized.
  - Cross-core reduction: the per-row sums are AllGather-ed (4.6 us floor
    vs AllReduce's 9.7+) in two halves - the first hides under the second
    matmul sweep - and summed locally with one strided DVE reduce.
  - Target path (concurrent with the main loop on GpSimd+DVE): rawdot,
    ||f||^2, ||wtgt||^2 per row; tgt = rawdot * exp(-0.5*ln(ssf*wn2));
    numerator = 64*(t*cos(m) - sqrt(1-t^2)*sin(m)) with sqrt via exp/ln
    (no trig tables needed). Combine ACT ops are dep-gated behind the last
    main-loop Exp so they cannot thrash the activation table mid-stream.
  - Final scalar on every core: -mean(num - ln(exp(num) + fullsum
    - exp(64*tgt))); core 0's value is returned.
"""
import math

import numpy as np
import ml_dtypes

import concourse.bass as bass
import concourse.tile as tile
from concourse import bacc, mybir
from concourse.bass_utils import run_bass_kernel_spmd
from concourse.tile import add_dep_helper

B = 2048          # batch
D = 512           # feature dim
C = 32768         # classes
NCORES = 8
CS = C // NCORES  # 4096 classes per core
S = 64.0
MARGIN = 0.5
EPS = 1e-7
COSM = math.cos(MARGIN)
SINM = math.sin(MARGIN)

NB = B // 128     # 16 batch tiles
NK = D // 128     # 4 contraction chunks
NCC = CS // 512   # 8 class chunks per core
NBC = B // 512    # 4 batch chunks (row-layout ops)

F32 = mybir.dt.float32
BF16 = mybir.dt.bfloat16
AF = mybir.ActivationFunctionType
ALU = mybir.AluOpType
BF16NP = ml_dtypes.bfloat16
FP8 = mybir.dt.float8e4
FP8NP = ml_dtypes.float8_e4m3fn

USE_FP8 = True
MMDT = FP8 if USE_FP8 else BF16
MMNP = FP8NP if USE_FP8 else BF16NP

_CACHE = {}

_ONE_SET = "natural_log_exp_and_others"


def _patch_act_tables():
    from concourse import hw_specs, bacc as bacc_mod
    if getattr(bacc_mod, "_act_tables_patched", False):
        return
    orig = hw_specs.get_activation_tables

    def patched(arch):
        t = orig(arch)
        return {name: (funcs if name == _ONE_SET else set())
                for name, funcs in t.items()}

    bacc_mod.get_activation_tables = patched
    bacc_mod._act_tables_patched = True


def _build():
    _patch_act_tables()
    nc = bacc.Bacc(None, target_bir_lowering=False, debug=False)

    fT_ext = nc.declare_dram_parameter("fT", [D, B], MMDT, isOutput=False)
    wT_ext = nc.declare_dram_parameter("wT", [D, CS], BF16, isOutput=False)
    fnat_ext = nc.declare_dram_parameter("fnat", [B, D], BF16, isOutput=False)
    wtgt_ext = nc.declare_dram_parameter("wtgt", [B, D], BF16, isOutput=False)
    out_ext = nc.declare_dram_parameter("out", [1, 1], F32, isOutput=True)

    ccA_in = nc.dram_tensor("ccA_in", [128, NB], F32)
    ccA_out = nc.dram_tensor("ccA_out", [128 * NCORES, NB], F32,
                             addr_space="Shared")
    ccB_in = nc.dram_tensor("ccB_in", [128, NB], F32)
    ccB_out = nc.dram_tensor("ccB_out", [128 * NCORES, NB], F32,
                             addr_space="Shared")

    with tile.TileContext(nc) as tc:
        with (
            tc.tile_pool(name="persist", bufs=1) as pp,
            tc.tile_pool(name="stream", bufs=4) as sp,
        ):
            # ---- persistent SBUF tiles ----
            wt3 = pp.tile([128, NK, CS], BF16)     # raw wT (bf16)
            whats = [pp.tile([128, NK, 512], MMDT, tag=f"what{i}",
                             name=f"what{i}")
                     for i in range(NCC)]          # normalized wT, per chunk
            ft3 = pp.tile([128, NK, B], MMDT)      # raw fT (fp8) = main lhsT
            fnat3 = pp.tile([128, NB, D], BF16)    # features, natural layout
            wtgt3 = pp.tile([128, NB, D], BF16)    # target weight rows
            ones_bf = pp.tile([128, 1], BF16)
            ones_f32 = pp.tile([128, 1], F32)
            ones_row = pp.tile([1, 128], BF16)
            sumsA = pp.tile([128, NB], F32)        # exp sums, cc 0-3
            sumsB = pp.tile([128, NB], F32)        # exp sums, cc 4-7
            rs_pt = pp.tile([128, NB], F32)        # 64/||f_b|| per-partition
            rawdot = pp.tile([128, NB], F32)
            ssf = pp.tile([128, NB], F32)
            wn2 = pp.tile([128, NB], F32)

            # ---- DMA the matmul operands in, split per k-chunk ----
            wTr = wT_ext[:].rearrange("(k p) c -> p k c", p=128)
            fTr = fT_ext[:].rearrange("(k p) b -> p k b", p=128)
            for n in range(NCC):
                for k in range(NK):
                    nc.sync.dma_start(wt3[:, k, bass.ts(n, 512)],
                                      wTr[:, k, bass.ts(n, 512)])
            for k in range(NK):
                nc.sync.dma_start(ft3[:, k, :], fTr[:, k, :])
            fnr = fnat_ext[:].rearrange("(t p) d -> p t d", p=128)
            for t in range(NB):
                nc.sync.dma_start(fnat3[:, t, :], fnr[:, t, :])
            nc.sync.dma_start(
                wtgt3[:], wtgt_ext[:].rearrange("(t p) d -> p t d", p=128))

            nc.vector.memset(ones_bf[:], 1.0)
            nc.vector.memset(ones_f32[:], 1.0)
            nc.vector.memset(ones_row[:], 1.0)

            pmain_cm = tc.tile_pool(name="pmain", bufs=2, space="PSUM")
            pmain = pmain_cm.__enter__()

            def norm_chunk(src3, col0, lhs_const, dst_slices):
                """rowsum -> 1/sqrt via exp(-ln/2) -> bcast -> scale.
                PSUM scratch borrows a main-loop z slot (bank 0: rowsum,
                bank 1: broadcast)."""
                zs = pmain.tile([128, 2048], F32, tag="z", name="zs")
                ps = zs[0:1, 0:512]
                for k in range(NK):
                    sq = sp.tile([128, 512], BF16, tag="sqt", name="sq")
                    nc.vector.tensor_mul(sq[:], src3[:, k, col0:col0 + 512],
                                         src3[:, k, col0:col0 + 512])
                    nc.tensor.matmul(ps, lhs_const[:], sq[:],
                                     start=(k == 0), stop=(k == NK - 1))
                lrow = sp.tile([1, 512], F32, tag="lrow", name="lrow")
                nc.scalar.activation(lrow[:], ps, AF.Ln)
                rnr = sp.tile([1, 512], BF16, tag="rnr", name="rnr")
                nc.scalar.activation(rnr[:], lrow[:], AF.Exp, scale=-0.5)
                pb = zs[:, 512:1024]
                nc.tensor.matmul(pb, ones_row[:], rnr[:],
                                 start=True, stop=True)
                bc = sp.tile([128, 512], BF16, tag="bc", name="bc")
                nc.vector.tensor_copy(bc[:], pb)
                last = None
                for k, dst in dst_slices:
                    last = nc.vector.tensor_mul(
                        dst, src3[:, k, col0:col0 + 512], bc[:])
                return last
            # per-row feature norms: ssf via ACT Square+accum, then
            # rs_pt[:, t] = 64/||f|| = exp(-0.5*ln(ssf/4096)); applied as the
            # per-partition scale of the main-loop Exp (b is the partition
            # axis of the matmul output), so features need no normalization
            # pass at all.
            def ssf_batch(h):
                for t in range(h, h + 8):
                    sqf = sp.tile([128, D], BF16, tag="sqf", name="sqf")
                    nc.scalar.activation(sqf[:], fnat3[:, t, :], AF.Square,
                                         accum_out=ssf[:, t:t + 1])
                lcol = sp.tile([128, 8], F32, tag="lcol", name="lcol")
                nc.scalar.activation(lcol[:], ssf[:, h:h + 8], AF.Ln,
                                     scale=1.0 / 4096.0)
                nc.scalar.activation(rs_pt[:, h:h + 8], lcol[:], AF.Exp,
                                     scale=-0.5)

            ssf_batch(0)
            # weight-col norms + normalized weight, per 512-chunk, paired
            # with the main-loop sweep that consumes them
            last_exp = None
            last_chunk = None
            for g, sums in ((0, sumsA), (1, sumsB)):
                for i in range(4):
                    n = 4 * g + i
                    last_chunk = norm_chunk(
                        wt3, 512 * n, ones_bf,
                        [(k, whats[n][:, k, :]) for k in range(NK)])
                if g == 0:
                    # second half of the row norms: needed from b=8, queued
                    # after the cc0-3 chunk rows so it can't delay them
                    ssf_batch(8)
                for b in range(NB):
                    zp = pmain.tile([128, 2048], F32, tag="z", name="zp")
                    for c4 in range(4):
                        cc = 4 * g + c4
                        if USE_FP8:
                            for j in range(NK // 2):
                                nc.tensor.matmul(
                                    zp[:, bass.ts(c4, 512)],
                                    ft3[:, 2 * j:2 * j + 2,
                                        bass.ts(b, 128)],
                                    whats[cc][:, 2 * j:2 * j + 2, :],
                                    start=(j == 0), stop=(j == 1),
                                    perf_mode=mybir.MatmulPerfMode.DoubleRow)
                        else:
                            for k in range(NK):
                                nc.tensor.matmul(
                                    zp[:, bass.ts(c4, 512)],
                                    ft3[:, k, bass.ts(b, 128)],
                                    whats[cc][:, k, :],
                                    start=(k == 0), stop=(k == NK - 1))
                    last_exp = nc.scalar.activation(
                        zp[:], zp[:], AF.Exp, scale=rs_pt[:, b:b + 1],
                        accum_out=sums[:, b:b + 1])
                if g == 0:
                    # first-half AllGather hides under the second half
                    nc.sync.dma_start(ccA_in[:], sumsA[:])
                    nc.gpsimd.collective_compute(
                        "AllGather", ALU.bypass,
                        replica_groups=[list(range(NCORES))],
                        ins=[ccA_in[:].opt()],
                        outs=[ccA_out[:].opt()],
                    )

            nc.sync.dma_start(ccB_in[:], sumsB[:])
            nc.gpsimd.collective_compute(
                "AllGather", ALU.bypass,
                replica_groups=[list(range(NCORES))],
                ins=[ccB_in[:].opt()],
                outs=[ccB_out[:].opt()],
            )
            # gather shards back and reduce across cores locally
            gathA = pp.tile([128, NCORES, NB], F32)
            nc.sync.dma_start(
                gathA[:], ccA_out[:].rearrange("(g p) c -> p g c", p=128))
            gathB = pp.tile([128, NCORES, NB], F32)
            nc.sync.dma_start(
                gathB[:], ccB_out[:].rearrange("(g p) c -> p g c", p=128))
            fullsumA = pp.tile([128, NB], F32)
            nc.vector.tensor_reduce(
                fullsumA[:], gathA[:].rearrange("p g c -> p c g"),
                axis=mybir.AxisListType.X, op=ALU.add)
            fullsumB = pp.tile([128, NB], F32)
            nc.vector.tensor_reduce(
                fullsumB[:], gathB[:].rearrange("p g c -> p c g"),
                axis=mybir.AxisListType.X, op=ALU.add)

            # ---- target path (concurrent with main loop; GpSimd + DVE) ----
            for t in range(NB):
                prod = sp.tile([128, D], BF16, tag="prod", name="prod")
                tm = nc.gpsimd.tensor_mul(prod[:], fnat3[:, t, :],
                                          wtgt3[:, t, :])
                if t == 0:
                    add_dep_helper(tm.ins, last_chunk.ins,
                                   reason="tgt path after norm prep")
                nc.vector.reduce_sum(rawdot[:, t:t + 1], prod[:],
                                     axis=mybir.AxisListType.X)
                sq2 = sp.tile([128, D], BF16, tag="prod", name="sq2")
                nc.gpsimd.tensor_mul(sq2[:], wtgt3[:, t, :], wtgt3[:, t, :])
                nc.vector.reduce_sum(wn2[:, t:t + 1], sq2[:],
                                     axis=mybir.AxisListType.X)

            # ---- combine: ACT ops gated behind the last main-loop Exp ----
            m2 = pp.tile([128, NB], F32)
            nc.vector.tensor_mul(m2[:], ssf[:], wn2[:])
            lm2 = pp.tile([128, NB], F32)
            ln_gate = nc.scalar.activation(lm2[:], m2[:], AF.Ln)
            add_dep_helper(ln_gate.ins, last_exp.ins,
                           reason="keep combine ACT ops after main-loop exps")
            rboth = pp.tile([128, NB], F32)
            nc.scalar.activation(rboth[:], lm2[:], AF.Exp, scale=-0.5)
            tgt = pp.tile([128, NB], F32)
            nc.vector.tensor_mul(tgt[:], rawdot[:], rboth[:])
            exptgt = pp.tile([128, NB], F32)
            nc.scalar.activation(exptgt[:], tgt[:], AF.Exp, scale=S)
            tclip = pp.tile([128, NB], F32)
            nc.vector.tensor_scalar(
                tclip[:], tgt[:], -1.0 + EPS, 1.0 - EPS,
                op0=ALU.max, op1=ALU.min)
            om = pp.tile([128, NB], F32)
            nc.vector.tensor_mul(om[:], tclip[:], tclip[:])
            nc.vector.tensor_scalar(om[:], om[:], -1.0, 1.0,
                                    op0=ALU.mult, op1=ALU.add)
            # sqrt(om) = exp(0.5*ln(om))
            lom = pp.tile([128, NB], F32)
            nc.scalar.activation(lom[:], om[:], AF.Ln)
            snt = pp.tile([128, NB], F32)
            nc.scalar.activation(snt[:], lom[:], AF.Exp, scale=0.5)
            num = pp.tile([128, NB], F32)
            nc.vector.tensor_scalar_mul(num[:], tclip[:], S * COSM)
            snts = pp.tile([128, NB], F32)
            nc.vector.tensor_scalar_mul(snts[:], snt[:], S * SINM)
            nc.vector.tensor_sub(num[:], num[:], snts[:])
            expnum = pp.tile([128, NB], F32)
            nc.scalar.activation(expnum[:], num[:], AF.Exp)

            # ---- final combine (identical on every core) ----
            fullsum = pp.tile([128, NB], F32)
            nc.vector.tensor_add(fullsum[:], fullsumA[:], fullsumB[:])
            denom = pp.tile([128, NB], F32)
            nc.vector.tensor_add(denom[:], expnum[:], fullsum[:])
            nc.vector.tensor_sub(denom[:], denom[:], exptgt[:])
            logd = pp.tile([128, NB], F32)
            nc.scalar.activation(logd[:], denom[:], AF.Ln)
            lvals = pp.tile([128, NB], F32)
            nc.vector.tensor_sub(lvals[:], num[:], logd[:])
            lred = pp.tile([128, 1], F32)
            nc.vector.reduce_sum(lred[:], lvals[:], axis=mybir.AxisListType.X)
            zf = pmain.tile([128, 2048], F32, tag="z", name="zf")
            nc.tensor.matmul(zf[0:1, 0:1], ones_f32[:], lred[:],
                             start=True, stop=True)
            outv = pp.tile([1, 1], F32)
            nc.scalar.mul(outv[:], zf[0:1, 0:1], -1.0 / float(B))
            nc.sync.dma_start(out_ext[:], outv[:])
            pmain_cm.__exit__(None, None, None)

    nc.compile()
    return nc


def _prep_inputs(features, y_true, weight):
    features = np.asarray(features, dtype=np.float32)
    weight = np.asarray(weight, dtype=np.float32)
    y = np.asarray(y_true).astype(np.int64)

    fT = features.T.astype(MMNP, order="C")            # [D, B]
    fnat = features.astype(BF16NP)                     # [B, D] bf16
    wtgt = weight[y].astype(BF16NP)                    # [B, D] bf16

    in_maps = []
    for i in range(NCORES):
        shard = weight[i * CS:(i + 1) * CS]            # [CS, D]
        wT = shard.T.astype(BF16NP, order="C")         # [D, CS]
        in_maps.append({"fT": fT, "wT": wT, "fnat": fnat, "wtgt": wtgt})
    return in_maps


def _run(features, y_true, weight, trace=False, **run_kwargs):
    if "nc" not in _CACHE:
        _CACHE["nc"] = _build()
    nc = _CACHE["nc"]
    in_maps = _prep_inputs(features, y_true, weight)
    res = run_bass_kernel_spmd(
        nc, in_maps, core_ids=list(range(NCORES)), trace=trace, **run_kwargs)
    out = np.asarray(res.results[0]["out"], dtype=np.float32)
    return np.float32(out.reshape(-1)[0]), res


def kernel(features, y_true, weight):
    val, _ = _run(features, y_true, weight, trace=False)
    return np.asarray(val, dtype=np.float32)
